# revision 12
# baseline (speedup 1.0000x reference)
"""Trainium2 Bass kernel for nn_BatchGraphEncoder (gnn_message_passing).

Math note: the reference's segment softmax uses B unique segment ids
(groups of size 1), so alpha == exp(x-x)/1 == 1.0 bit-exactly for any
finite scores.  The output is therefore independent of the attention
inputs (w_i, w_j, w_k) and reduces to pure batch sums:

    out[:,   0:128] = sum_b h[b,:]      (broadcast over the N=512 rows)
    out[:, 128:256] = sum_b r[b,:]      (broadcast)
    out[:, 256:384] = sum_b t[b,:,:]    ([512, 128])

This is a memory-bound reduction over B=2048 dominated by reading t
(512 MB).  Strategy: shard B across the 8 cores (data parallel),
reduce over the local batch on-device, and sum the 8 partials on the
host.

Layout: the [B_loc, 65536] shard is viewed as [NCH, 128, 4096] so each
8-batch-row chunk IS a [128, 4096] DRAM slab: partition p = 16*bi + q
holds row (8k+bi), flat columns [4096q, 4096q+4096).  Each chunk DMA is
a plain 2 MB block copy with 16 KB contiguous runs - 128 descriptors
per chunk.  Descriptor size matters a lot here: 2 KB runs cap the
per-core stream at ~379 GB/s while 16 KB runs reach ~420 GB/s (near
the 435 GB/s SBUF-AXI fabric ceiling).  The 8 batch rows of a chunk
land on disjoint partition groups, so the on-device reduction is ONE
tensor_tensor add per chunk (acc += tile, 4096 lane-cycles = 4.3 us
vs the ~4.8 us chunk DMA - the DVE never backlogs, which killed the
~20 us fold-drain tail of the earlier fold-tree design).  The final
8-group cross-partition sum (acc.reshape(8,16,4096).sum(0)) moves to
the host along with the 8-core partial sum - out_t_part is [128, 4096]
(2 MB) per core, written as four column-quarter DMAs interleaved with
the last chunk's quarter-merges to trim the tail.

The whole t stream rides the SP HWDGE ring so chunks complete strictly
in order (two alternating rings deliver chunks in near-simultaneous
pairs, doubling the post-stream merge tail); the ACT ring carries only
the tiny h/r result and half of the output quarters.  h/r sums ride
the otherwise-idle TensorEngine (ones-column stationary -> PSUM rows
0/1) with SWDGE (gpsimd) loads, finalized mid-stream so nothing trails
the t stream.

Load balancing: per-core bandwidth episodes rove between cores on a
minutes timescale, but a day of profiling shows a stable asymmetry:
odd physical NCs hold ~417 GB/s in virtually every run while even
physical NCs (logical 0/2/4/6 here) drop to 330-360 GB/s in frequent
episodes, phys NC 0 (logical 6) nearly always.  Shards are sized for
those reliability classes.  Rows a core does not own are skipped via
conditional DMAs (sync-engine pid predicate) and their stale-buffer
merges are gated by per-partition masks.
"""

import numpy as np

B, N, D = 2048, 512, 128
NCORES = 8
FLAT = N * D                 # 65536 flattened (n, d) columns
CW = 4096                    # chunk free width = 16 KB descriptor runs
RPC = 8                      # batch rows per chunk (8 * 16 partitions)

# Per-core row counts, proportional to observed reliability: odd phys
# NCs (logical 1, 3, 5, 7) sustain ~417 GB/s in virtually every run;
# even phys NCs (logical 0, 2, 4, 6) drop to 330-360 GB/s in frequent
# episodes, with phys NC 0 (logical 6) the worst and most persistent.
SIZES = [240, 280, 240, 272, 240, 280, 216, 280]
B_MAX = max(SIZES)           # 264
NCH = B_MAX // RPC           # 33 chunks of [128, 4096] = 2 MB
assert sum(SIZES) == B and all(s % RPC == 0 for s in SIZES)

# chunk c covers rows [8c, 8c+8); cores with SIZES <= 8c skip it.
_SKIP = {c: tuple(p for p in range(NCORES) if SIZES[p] <= RPC * c)
         for c in range(NCH)}
COND = {c: s for c, s in _SKIP.items() if s}          # chunk -> skip pids
TIERS = sorted(set(COND.values()))                    # distinct skip sets

# Emission order: first 8 and the last slots unconditional (a
# conditional chunk must never be a pool buffer's first use - the
# masked merge of a skipped DMA would read uninitialized SBUF),
# conditionals mid-stream.
_UNC = [c for c in range(NCH) if c not in COND]
_CND = sorted(COND)
ORDER = list(_UNC[:8])
_rest = _UNC[8:]
for i, c in enumerate(_CND):
    ORDER.append(c)
    ORDER.extend(_rest[2 * i: 2 * i + 2])
ORDER.extend(_rest[2 * len(_CND):])
assert sorted(ORDER) == list(range(NCH))
assert all(c not in COND for c in ORDER[:8]) and ORDER[-1] not in COND

_BUILT = None
# test.py can inject {"trace": True, ...} here; harness path leaves it empty.
RUN_KWARGS = {}
LAST_RESULTS = None


def _build():
    from concourse import bacc, tile, mybir

    f32 = mybir.dt.float32
    add = mybir.AluOpType.add
    nc = bacc.Bacc(
        "TRN2",
        target_bir_lowering=False,
        debug=False,
        enable_asserts=False,
        num_devices=NCORES,
    )
    t_in = nc.dram_tensor("t_shard", [NCH, 128, CW], f32,
                          kind="ExternalInput").ap()
    h_in = nc.dram_tensor("h_shard", [B_MAX, D], f32, kind="ExternalInput").ap()
    r_in = nc.dram_tensor("r_shard", [B_MAX, D], f32, kind="ExternalInput").ap()
    out_t = nc.dram_tensor("out_t_part", [128, CW], f32,
                           kind="ExternalOutput").ap()
    out_hr = nc.dram_tensor("out_hr_part", [2, D], f32, kind="ExternalOutput").ap()

    with tile.TileContext(nc) as tc:
        with (
            tc.tile_pool(name="wconst", bufs=1) as wpool,
            tc.tile_pool(name="loads", bufs=8) as loads,
            tc.tile_pool(name="hr", bufs=6) as hrpool,
            tc.tile_pool(name="res", bufs=1) as res,
            tc.tile_pool(name="acc", bufs=1, space="PSUM") as ppool,
        ):
            W = wpool.tile([128, 256], f32)
            masks = {
                t: wpool.tile([128, 1], f32, name=f"mask{i}")
                for i, t in enumerate(TIERS)
            }
            psum_hr = ppool.tile([128, D], f32)
            acc = res.tile([128, CW], f32)
            skip_cond = {}

            def emit_setup_and_hr():
                # Emitted after the first few t loads so the pid
                # register loads and h/r DMAs never delay the t
                # stream's start; h/r loads ride the SWDGE (gpsimd)
                # ring, keeping the SP HWDGE ring exclusively on t.
                # W is zero except column 128 == 1.0; W[:, 128-j:256-j]
                # is a [128, 128] stationary whose column j is all-ones.
                nc.vector.memset(W[:], 0.0)
                nc.vector.memset(W[:, 128:129], 1.0)
                # masks[t] = 0.0 on cores that skip tier t, 1.0
                # elsewhere; they gate the accumulator merges of
                # conditional chunks.
                for m in masks.values():
                    nc.vector.memset(m[:], 1.0)
                pid_vec = nc.vector.partition_id()
                for p in sorted({p for s in TIERS for p in s}):
                    with tc.If(pid_vec == p):
                        for t in TIERS:
                            if p in t:
                                nc.vector.memset(masks[t][:], 0.0)
                pid_sync = nc.sync.partition_id()
                for t in TIERS:
                    cs = None
                    for p in t:
                        es = pid_sync != p
                        cs = es if cs is None else cs * es
                    skip_cond[t] = cs

                # h / r batch sums -> rows 0 / 1 of psum_hr
                # (padding rows beyond a core's shard are zeros: exact)
                chunks = []
                for row, src in ((0, h_in), (1, r_in)):
                    for c0 in range(0, B_MAX, 128):
                        k = min(128, B_MAX - c0)
                        ht = hrpool.tile([128, D], f32, name=f"ht{row}_{c0}")
                        nc.gpsimd.dma_start(ht[:k, :], src[c0: c0 + k, :])
                        chunks.append((row, ht, k))
                for i, (row, ht, k) in enumerate(chunks):
                    nc.tensor.matmul(
                        psum_hr[:],
                        W[:k, 128 - row: 256 - row],
                        ht[:k, :],
                        start=(i == 0),
                        stop=(i == len(chunks) - 1),
                    )

            # --- t batch sum: one DVE add per 2 MB chunk ---
            for k, c in enumerate(ORDER):
                if k == 3:
                    emit_setup_and_hr()
                if k == 14:
                    # finalize h/r mid-stream so nothing trails the tail
                    res_hr = res.tile([2, D], f32)
                    nc.vector.tensor_copy(res_hr[:], psum_hr[0:2, :])
                    nc.scalar.dma_start(out_hr[:], res_hr[:])
                tl = loads.tile([128, CW], f32)
                if c in COND:
                    # Skipped on cores not owning these rows: the slot
                    # then holds stale (finite) data from an earlier
                    # chunk; the masked merge zeroes it.
                    nc.sync.dma_start(tl[:], t_in[c], cond=skip_cond[COND[c]])
                else:
                    nc.sync.dma_start(tl[:], t_in[c])
                if k == 0:
                    nc.vector.tensor_copy(acc[:], tl[:])
                elif c in COND:
                    # acc = (tile * mask) + acc
                    nc.vector.scalar_tensor_tensor(
                        acc[:], tl[:], masks[COND[c]][:], acc[:],
                        mybir.AluOpType.mult, add,
                    )
                elif k == NCH - 1:
                    # last chunk: merge in column quarters into FRESH
                    # tiles (Tile deps are whole-tile granular - merging
                    # in place would make every output DMA wait for the
                    # last merge); each quarter's output DMA
                    # (alternating rings) overlaps the remaining merges
                    q = CW // 4
                    for j in range(4):
                        sl = slice(j * q, (j + 1) * q)
                        rq = res.tile([128, q], f32, name=f"res_q{j}")
                        nc.vector.tensor_tensor(
                            rq[:], acc[:, sl], tl[:, sl], add)
                        out_dma = nc.sync if j % 2 == 0 else nc.scalar
                        out_dma.dma_start(out_t[:, sl], rq[:])
                else:
                    nc.vector.tensor_tensor(acc[:], acc[:], tl[:], add)

    nc.compile()
    return nc


def _get_built():
    global _BUILT
    if _BUILT is None:
        _BUILT = _build()
    return _BUILT


def kernel(h, r, t, w_i, w_j, w_k):
    global LAST_RESULTS
    from concourse import bass_utils

    nc = _get_built()
    t2 = np.ascontiguousarray(t, dtype=np.float32).reshape(B, FLAT)
    h = np.ascontiguousarray(h, dtype=np.float32)
    r = np.ascontiguousarray(r, dtype=np.float32)

    def pad(a, ncols):
        out = np.zeros((B_MAX, ncols), dtype=np.float32)
        out[: a.shape[0]] = a
        return out

    starts = np.concatenate([[0], np.cumsum(SIZES)])
    in_maps = []
    for c in range(NCORES):
        s, e = int(starts[c]), int(starts[c + 1])
        tc_ = t2[s:e] if e - s == B_MAX else pad(t2[s:e], FLAT)
        hc = h[s:e] if e - s == B_MAX else pad(h[s:e], D)
        rc = r[s:e] if e - s == B_MAX else pad(r[s:e], D)
        in_maps.append(
            {
                "t_shard": tc_.reshape(NCH, 128, CW),
                "h_shard": hc,
                "r_shard": rc,
            }
        )
    results = bass_utils.run_bass_kernel_spmd(
        nc, in_maps, core_ids=list(range(NCORES)), **RUN_KWARGS
    )
    LAST_RESULTS = results

    # acc[16*bi + q, c] = sum_k t_loc[8k+bi, 4096q+c]; finish the sum over
    # cores and over bi on the host (f64), then unflatten to [N, D].
    sum_t = np.zeros((128, CW), dtype=np.float64)
    sum_h = np.zeros(D, dtype=np.float64)
    sum_r = np.zeros(D, dtype=np.float64)
    for c in range(NCORES):
        sum_t += results.results[c]["out_t_part"]
        sum_h += results.results[c]["out_hr_part"][0]
        sum_r += results.results[c]["out_hr_part"][1]
    t_full = sum_t.reshape(RPC, 16, CW).sum(axis=0).reshape(N, D)

    out = np.empty((N, 3 * D), dtype=np.float32)
    out[:, 0:D] = sum_h.astype(np.float32)[None, :]
    out[:, D: 2 * D] = sum_r.astype(np.float32)[None, :]
    out[:, 2 * D:] = t_full.astype(np.float32)
    return out
